# revision 1
# baseline (speedup 1.0000x reference)
"""Trainium2 Bass kernel for a dense transformer block (nn_Block_5360119185758).

B=4, T=2048, C=512, H=8, DH=64, FF=2048, causal attention, fp32 I/O.

Sharding: 8 cores = (batch b, half) pairs; zero collectives. Each core
computes K/V for its full batch but queries/proj/FFN only for its half of
the rows (alternating 128-row blocks, for causal load balance). Columns are
host-permuted so each core's own rows come first -> the device program is
identical across cores (SPMD); all core-dependence lives in the input data.

On-device layout is feature-major ([C, T] transposed) end-to-end: no
on-device transposes. Matmuls run in float32r (TF32-like, 1 cyc/row).
LayerNorm gains/biases are folded into the consumer weights host-side.
Softmax denominators come from a ones-column appended to V (M=65 AV
matmuls). Causal structure is handled by skipping k-blocks beyond the
diagonal plus additive boundary masks (passed as per-core input data).
"""

import sys

if "/opt/trn_rl_repo" not in sys.path:
    sys.path.insert(0, "/opt/trn_rl_repo")

import numpy as np

import concourse.bass as bass
import concourse.mybir as mybir
import concourse.tile as tile
from concourse import bacc
from concourse.bass_utils import run_bass_kernel_spmd

F32 = mybir.dt.float32
F32R = mybir.dt.float32r
AF = mybir.ActivationFunctionType
ALU = mybir.AluOpType

B, T, C, H, DH, FF = 4, 2048, 512, 8, 64, 4 * 512
P = 128
KC = C // P            # 4 c-chunks
NBLK = T // P          # 16 global t-blocks
TQ = T // 2            # 1024 own rows per core
NJ = TQ // 256         # 4 supertiles of 256 own cols
FC = FF // P           # 16 f-chunks
EPS = 1e-5
NEG = -1.0e9


def _build_nc(phases=3.0):
    nc = bacc.Bacc(None, target_bir_lowering=False)

    xT = nc.dram_tensor("xT", [C, T], F32, kind="ExternalInput")
    wq = nc.dram_tensor("wq", [C, C], F32, kind="ExternalInput")
    wk = nc.dram_tensor("wk", [C, C], F32, kind="ExternalInput")
    wv = nc.dram_tensor("wv", [C, C], F32, kind="ExternalInput")
    wp = nc.dram_tensor("wp", [C, C], F32, kind="ExternalInput")
    w1 = nc.dram_tensor("w1", [C, FF], F32, kind="ExternalInput")
    w2 = nc.dram_tensor("w2", [FF, C], F32, kind="ExternalInput")
    bqd = nc.dram_tensor("bq", [C], F32, kind="ExternalInput")
    bkd = nc.dram_tensor("bk", [C], F32, kind="ExternalInput")
    bvd = nc.dram_tensor("bv", [C], F32, kind="ExternalInput")
    bpd = nc.dram_tensor("bp", [C], F32, kind="ExternalInput")
    b1d = nc.dram_tensor("b1", [FF], F32, kind="ExternalInput")
    b2d = nc.dram_tensor("b2", [C], F32, kind="ExternalInput")
    maskd = nc.dram_tensor("mask", [4, P, 256], F32, kind="ExternalInput")
    consts = nc.dram_tensor("consts", [P, 160], F32, kind="ExternalInput")
    outT = nc.dram_tensor("outT", [C, TQ], F32, kind="ExternalOutput")

    with tile.TileContext(nc) as tc:
        _emit(nc, tc, xT, wq, wk, wv, wp, w1, w2,
              bqd, bkd, bvd, bpd, b1d, b2d, maskd, consts, outT, phases=phases)
    nc.compile()
    return nc


def _emit(nc, tc, xT, wq, wk, wv, wp, w1, w2,
          bqd, bkd, bvd, bpd, b1d, b2d, maskd, consts, outT, phases=3):
    import contextlib
    ctx = contextlib.ExitStack()
    with ctx:
        res = ctx.enter_context(tc.tile_pool(name="res", bufs=1))

        # --- resident weights (lhsT / rhs layouts share [P, KC, C]) ---
        def load_w(dram, shape, name):
            t = res.tile(shape, F32R, name=name, tag=name)
            nc.sync.dma_start(
                t[:], dram.rearrange("(kc p) n -> p kc n", p=P).bitcast(F32R))
            return t

        wk_s = load_w(wk, [P, KC, C], "wk_s")
        wv_s = load_w(wv, [P, KC, C], "wv_s")
        wq_s = load_w(wq, [P, KC, C], "wq_s")

        def load_b(dram, n, name):
            t = res.tile([P, n], F32, name=name, tag=name)
            nc.sync.dma_start(t[:], dram.rearrange("(mc p) -> p mc", p=P))
            return t

        bq_s = load_b(bqd, KC, "bq_s")
        bk_s = load_b(bkd, KC, "bk_s")
        bv_s = load_b(bvd, KC, "bv_s")
        bp_s = load_b(bpd, KC, "bp_s")
        b1_s = load_b(b1d, FC, "b1_s")
        b2_s = load_b(b2d, KC, "b2_s")

        mask_s = res.tile([P, 4, 256], F32)
        nc.sync.dma_start(mask_s[:], maskd.rearrange("m p t -> p m t"))

        ones_l = res.tile([P, 1], F32R)
        nc.sync.dma_start(ones_l[:], consts[:, 0:1].bitcast(F32R))
        eps_r = res.tile([1, 1], F32)
        nc.sync.dma_start(eps_r[:], consts[0:1, 1:2])

        # --- resident activations ---
        KT = res.tile([P, KC, T], F32R)       # K^T feature-major, all heads
        VO = res.tile([P, NBLK, H, DH + 1], F32R)  # V token-major + ones col
        QT = res.tile([P, KC, TQ], F32R)      # Q^T for own cols
        nc.sync.dma_start(
            VO[:, :, :, DH],
            consts[:, 2:2 + NBLK * H].rearrange("p (a b) -> p a b", a=NBLK)
            .bitcast(F32R))

        # ---------------- Phase A: LN1 stats + QKV projections ----------
        with (
            tc.tile_pool(name="pa", bufs=2) as pa,
            tc.tile_pool(name="pa1", bufs=2) as pa1,
            tc.tile_pool(name="ppa", bufs=3, space="PSUM") as ppa,
            tc.tile_pool(name="pst", bufs=2, space="PSUM") as pst,
        ):
            for ts in range(4):          # 512-col slabs of permuted T
                tsl = slice(ts * 512, (ts + 1) * 512)
                xt = pa.tile([P, KC, 512], F32R, tag="xt")
                nc.sync.dma_start(
                    xt[:], xT.rearrange("(kc p) t -> p kc t", p=P)[:, :, tsl]
                    .bitcast(F32R))

                ssum = pst.tile([1, 512], F32, tag="st")
                ssq = pst.tile([1, 512], F32, tag="st")
                for kc in range(KC):
                    nc.tensor.matmul(ssum[:], ones_l[:], xt[:, kc, :],
                                     start=(kc == 0), stop=(kc == KC - 1))
                for kc in range(KC):
                    xsq = pa1.tile([P, 512], F32R, tag="xsq")
                    nc.vector.tensor_mul(xsq[:], xt[:, kc, :], xt[:, kc, :])
                    nc.tensor.matmul(ssq[:], ones_l[:], xsq[:],
                                     start=(kc == 0), stop=(kc == KC - 1))

                mu = pa1.tile([1, 512], F32, tag="mu")
                r = pa1.tile([1, 512], F32, tag="r")
                sd = pa1.tile([1, 512], F32, tag="sd")
                nc.scalar.copy(mu[:], ssum[:])
                nc.vector.tensor_mul(sd[:], mu[:], mu[:])      # mu^2
                nc.vector.tensor_tensor(sd[:], ssq[:], sd[:], ALU.subtract)
                nc.scalar.activation(sd[:], sd[:], AF.Sqrt, bias=eps_r[:])
                nc.vector.reciprocal(r[:], sd[:])

                mub = pa1.tile([P, 512], F32, tag="mub")
                rb = pa1.tile([P, 512], F32, tag="rb")
                nc.gpsimd.partition_broadcast(mub[:], mu[:])
                nc.gpsimd.partition_broadcast(rb[:], r[:])
                xh = pa.tile([P, KC, 512], F32R, tag="xh")
                for kc in range(KC):
                    nc.vector.tensor_tensor(
                        xh[:, kc, :], xt[:, kc, :], mub[:], ALU.subtract)
                    nc.vector.tensor_tensor(
                        xh[:, kc, :], xh[:, kc, :], rb[:], ALU.mult)

                # K^T (+ Q^T for own slabs): lhsT=W, rhs=xhat
                plans = [(wk_s, bk_s, KT[:, :, tsl])]
                if ts < 2:
                    plans.append(
                        (wq_s, bq_s, QT[:, :, slice(ts * 512, ts * 512 + 512)]))
                for w_s, b_s, dst in plans:
                    for mc in range(KC):
                        ps = ppa.tile([P, 512], F32, tag="mmA")
                        for kc in range(KC):
                            nc.tensor.matmul(
                                ps[:], w_s[:, kc, mc * P:(mc + 1) * P],
                                xh[:, kc, :],
                                start=(kc == 0), stop=(kc == KC - 1))
                        nc.scalar.activation(
                            dst[:, mc, :], ps[:], AF.Identity,
                            bias=b_s[:, mc:mc + 1])

                # V token-major: lhsT=xhat, rhs=Wv  -> [t, (h d)]
                for tm in range(4):
                    ps = ppa.tile([P, 512], F32, tag="mmA")
                    for kc in range(KC):
                        nc.tensor.matmul(
                            ps[:], xh[:, kc, tm * P:(tm + 1) * P],
                            wv_s[:, kc, :],
                            start=(kc == 0), stop=(kc == KC - 1))
                    nc.vector.tensor_copy(
                        VO[:, ts * 4 + tm, :, 0:DH],
                        ps[:].rearrange("p (h d) -> p h d", h=H))

        # phase-B-only weights: queued after the phase-A DMA stream so the
        # first xT slab isn't stuck behind 5MB of FFN weights
        wp_s = load_w(wp, [P, KC, C], "wp_s")
        w1_s = load_w(w1, [P, KC, FF], "w1_s")
        if phases < 2:
            return
        # ---------------- Phase B: attention + proj + LN2 + FFN ---------
        with (
            tc.tile_pool(name="pb", bufs=1) as pb,
            tc.tile_pool(name="pb2", bufs=1) as pb2,
            tc.tile_pool(name="pbq", bufs=1) as pbq,
            tc.tile_pool(name="pbr", bufs=1) as pbr,
            tc.tile_pool(name="ppt", bufs=3) as ppt,
            tc.tile_pool(name="pb1", bufs=2) as pb1,
            tc.tile_pool(name="w2p", bufs=3) as w2p,
            tc.tile_pool(name="pps", bufs=8, space="PSUM") as pps,
        ):
            for j in range(NJ):
                jsl = slice(j * 256, (j + 1) * 256)
                nown = 2 * j + 2          # own s-blocks (last 2 masked)
                # s-block list: own then partner, boundary-masked last two of
                # each get mask slots 0,1 (own) / 2,3 (partner)
                sblocks = [(m, (0 if m == nown - 2 else 1 if m == nown - 1
                                else None))
                           for m in range(nown)]
                sblocks += [(8 + m, (2 if m == nown - 2 else
                                     3 if m == nown - 1 else None))
                            for m in range(nown)]
                ns = len(sblocks)

                x_own = pb.tile([P, KC, 256], F32, tag="xo")
                nc.sync.dma_start(
                    x_own[:],
                    xT.rearrange("(kc p) t -> p kc t", p=P)[:, :, jsl])

                qe = pbq.tile([P, KC, 256], F32R, tag="qe")
                qo = pbq.tile([P, KC, 256], F32R, tag="qo")
                nc.vector.tensor_copy(qe[0:64, :, :], QT[0:64, :, jsl])
                nc.vector.tensor_scalar(
                    qe[64:P, :, :], QT[64:P, :, jsl], 0.0, None, ALU.mult)
                nc.vector.tensor_copy(qo[64:P, :, :], QT[64:P, :, jsl])
                nc.vector.tensor_scalar(
                    qo[0:64, :, :], QT[0:64, :, jsl], 0.0, None, ALU.mult)
                OT = pb2.tile([P, KC, 256], F32R, tag="ot")
                for hp in range(H // 2):      # head pairs share sc tiles
                    av = [None, None]
                    pending = None            # (pt, ki) awaiting AV matmuls
                    for ki in range(ns):
                        sb, mi = sblocks[ki]
                        ssl = slice(sb * P, (sb + 1) * P)
                        sc = pps.tile([P, 512], F32, tag="u", name="sc")
                        for o in range(2):   # even/odd head of pair
                            nc.tensor.matmul(
                                sc[:, o * 256:(o + 1) * 256],
                                KT[:, hp, ssl],
                                (qe if o == 0 else qo)[:, hp, :],
                                start=True, stop=True)
                        if mi is not None:
                            m2 = mask_s[:, mi, None, :] \
                                .to_broadcast((P, 2, 256))
                            nc.vector.tensor_tensor(
                                sc[:].rearrange("p (b t) -> p b t", b=2),
                                sc[:].rearrange("p (b t) -> p b t", b=2),
                                m2, ALU.add)
                        pt = ppt.tile([P, 512], F32R, tag="pt", name="pt")
                        nc.scalar.activation(pt[:], sc[:], AF.Exp,
                                             scale=float(1.0 / np.sqrt(DH)))
                        for o in range(2):
                            if av[o] is None:
                                av[o] = pps.tile([P, 512], F32,
                                                 tag="u", name="av")
                        if pending is not None:
                            ppt_, pki = pending
                            psb = sblocks[pki][0]
                            for o in range(2):
                                nc.tensor.matmul(
                                    av[o][0:DH + 1, 0:256],
                                    VO[:, psb, 2 * hp + o, :],
                                    ppt_[:, o * 256:(o + 1) * 256],
                                    start=(pki == 0), stop=False)
                        pending = (pt, ki)
                    ppt_, pki = pending
                    psb = sblocks[pki][0]
                    for o in range(2):
                        nc.tensor.matmul(
                            av[o][0:DH + 1, 0:256],
                            VO[:, psb, 2 * hp + o, :],
                            ppt_[:, o * 256:(o + 1) * 256],
                            start=(pki == 0), stop=(pki == ns - 1))
                    for o in range(2):
                        rec = pbr.tile([1, 256], F32, tag="rec")
                        nc.vector.reciprocal(rec[:], av[o][DH:DH + 1, 0:256])
                        recb = pb1.tile([DH, 256], F32, tag="recb")
                        nc.gpsimd.partition_broadcast(recb[:], rec[:])
                        h0 = 64 * o
                        dst = OT[h0:h0 + 64, hp, :]
                        nc.vector.tensor_tensor(
                            dst, av[o][0:DH, 0:256], recb[:], ALU.mult)
                        nc.vector.tensor_scalar(
                            dst, dst, bv_s[h0:h0 + 64, hp:hp + 1], None,
                            ALU.add)

                if phases < 2.2:
                    continue
                # proj + residual -> resid1 (feature-major)
                resid = pb2.tile([P, KC, 256], F32R, tag="resid")
                for cc in range(KC):
                    ps = pps.tile([P, 512], F32, tag="u", name="ps")
                    for kc in range(KC):
                        nc.tensor.matmul(
                            ps[:, 0:256], wp_s[:, kc, cc * P:(cc + 1) * P],
                            OT[:, kc, :],
                            start=(kc == 0), stop=(kc == KC - 1))
                    nc.vector.tensor_scalar(
                        resid[:, cc, :], ps[:, 0:256], bp_s[:, cc:cc + 1],
                        None, ALU.add)
                    nc.vector.tensor_tensor(
                        resid[:, cc, :], resid[:, cc, :], x_own[:, cc, :],
                        ALU.add)

                # LN2 stats
                ssum = pps.tile([1, 512], F32, tag="u", name="ssum")
                ssq = pps.tile([1, 512], F32, tag="u", name="ssq")
                for kc in range(KC):
                    nc.tensor.matmul(ssum[0:1, 0:256], ones_l[:],
                                     resid[:, kc, :],
                                     start=(kc == 0), stop=(kc == KC - 1))
                for kc in range(KC):
                    xsq = pb1.tile([P, 256], F32R, tag="xsq2")
                    nc.vector.tensor_mul(xsq[:], resid[:, kc, :],
                                         resid[:, kc, :])
                    nc.tensor.matmul(ssq[0:1, 0:256], ones_l[:], xsq[:],
                                     start=(kc == 0), stop=(kc == KC - 1))
                mu = pbr.tile([1, 256], F32, tag="mu2")
                r = pbr.tile([1, 256], F32, tag="r2")
                sd = pbr.tile([1, 256], F32, tag="sd2")
                nc.scalar.copy(mu[:], ssum[0:1, 0:256])
                nc.vector.tensor_mul(sd[:], mu[:], mu[:])
                nc.vector.tensor_tensor(sd[:], ssq[0:1, 0:256], sd[:],
                                        ALU.subtract)
                nc.scalar.activation(sd[:], sd[:], AF.Sqrt, bias=eps_r[:])
                nc.vector.reciprocal(r[:], sd[:])

                if phases < 2.5:
                    continue
                mub2 = pb1.tile([P, 256], F32, tag="mub2")
                rb2 = pb1.tile([P, 256], F32, tag="rb2")
                nc.gpsimd.partition_broadcast(mub2[:], mu[:])
                nc.gpsimd.partition_broadcast(rb2[:], r[:])
                xh2 = pb2.tile([P, KC, 256], F32R, tag="xh2")
                for kc in range(KC):
                    nc.vector.tensor_tensor(
                        xh2[:, kc, :], resid[:, kc, :], mub2[:], ALU.subtract)
                    nc.vector.tensor_tensor(
                        xh2[:, kc, :], xh2[:, kc, :], rb2[:], ALU.mult)

                # FFN: f-outer, W2 streamed; FFN2 accumulates in one 2-bank
                # pair tile [P, 2, 512] = 4 x 256-wide c' chunks
                if phases < 2.8:
                    continue
                f2t = [pps.tile([P, 512], F32, tag="u", name="f2")
                       for _ in range(KC)]
                f2sl = [t[:, 0:256] for t in f2t]
                pend = None               # (rl, w2f, fc) awaiting ffn2
                for fc in range(FC):
                    ps = pps.tile([P, 512], F32, tag="u", name="ps")
                    for kc in range(KC):
                        nc.tensor.matmul(
                            ps[:, 0:256], w1_s[:, kc, fc * P:(fc + 1) * P],
                            xh2[:, kc, :],
                            start=(kc == 0), stop=(kc == KC - 1))
                    rl = pb1.tile([P, 256], F32R, tag="rl")
                    nc.scalar.activation(rl[:], ps[:, 0:256], AF.Relu,
                                         bias=b1_s[:, fc:fc + 1])
                    w2f = w2p.tile([P, C], F32R, tag="w2f")
                    nc.sync.dma_start(
                        w2f[:], w2[fc * P:(fc + 1) * P, :].bitcast(F32R))
                    if pend is not None:
                        prl, pw2f, pfc = pend
                        for cc in range(KC):
                            nc.tensor.matmul(
                                f2sl[cc],
                                pw2f[:, cc * P:(cc + 1) * P], prl[:],
                                start=(pfc == 0), stop=False)
                    pend = (rl, w2f, fc)
                prl, pw2f, pfc = pend
                for cc in range(KC):
                    nc.tensor.matmul(
                        f2sl[cc],
                        pw2f[:, cc * P:(cc + 1) * P], prl[:],
                        start=(pfc == 0), stop=(pfc == FC - 1))

                ot = pb2.tile([P, KC, 256], F32, tag="outb")
                for cc in range(KC):
                    nc.vector.tensor_scalar(
                        ot[:, cc, :], f2sl[cc],
                        b2_s[:, cc:cc + 1], None, ALU.add)
                    nc.vector.tensor_tensor(
                        ot[:, cc, :], ot[:, cc, :], resid[:, cc, :], ALU.add)
                nc.sync.dma_start(
                    outT.rearrange("(kc p) t -> p kc t", p=P)[:, :, jsl],
                    ot[:])


_NC_CACHE = None


def _get_nc():
    global _NC_CACHE
    if _NC_CACHE is None:
        _NC_CACHE = _build_nc()
    return _NC_CACHE


def _perm_blocks(half):
    return list(range(half, NBLK, 2)) + list(range(1 - half, NBLK, 2))


def _make_mask(half):
    m = np.zeros((4, P, 256), np.float32)
    s_in = np.arange(P)[:, None]
    t_in = np.arange(256)[None, :] % P
    n = np.arange(256)[None, :] // P        # own t-block 0/1 (relative)
    g_t = 2 * n + half
    for mi in range(4):
        if mi < 2:
            g_s = 2 * mi + half             # own s-block (relative)
        else:
            g_s = 2 * (mi - 2) + 1 - half   # partner s-block
        allowed = (g_s * P + s_in) <= (g_t * P + t_in)
        m[mi][~allowed] = NEG
    return m


def kernel(x, ln1_g, ln1_b, Wq, bq, Wk, bk, Wv, bv, Wp, bp,
           ln2_g, ln2_b, W1, b1, W2, b2):
    x = np.asarray(x, np.float32)
    f = lambda a: np.asarray(a, np.float32)
    ln1_g, ln1_b, ln2_g, ln2_b = f(ln1_g), f(ln1_b), f(ln2_g), f(ln2_b)
    Wqf = f(Wq).transpose(1, 0, 2).reshape(C, C)
    Wkf = f(Wk).transpose(1, 0, 2).reshape(C, C)
    Wvf = f(Wv).transpose(1, 0, 2).reshape(C, C)
    wq_e = np.ascontiguousarray(ln1_g[:, None] * Wqf)
    wk_e = np.ascontiguousarray(ln1_g[:, None] * Wkf)
    wv_e = np.ascontiguousarray(ln1_g[:, None] * Wvf)
    bq_e = f(bq).reshape(C) + ln1_b @ Wqf
    bk_e = f(bk).reshape(C) + ln1_b @ Wkf
    bv_e = f(bv).reshape(C) + ln1_b @ Wvf
    w1_e = np.ascontiguousarray(ln2_g[:, None] * f(W1))
    b1_e = f(b1) + ln2_b @ f(W1)
    wp_e, bp_e, w2_e, b2_e = f(Wp), f(bp), f(W2), f(b2)

    nc = _get_nc()
    consts_np = np.ones((P, 160), np.float32)
    consts_np[:, 0] = 1.0 / C
    consts_np[0, 1] = EPS
    in_maps = []
    for core in range(8):
        b, half = divmod(core, 2)
        pb_ = _perm_blocks(half)
        xp = x[b].reshape(NBLK, P, C)[pb_].reshape(T, C)
        in_maps.append({
            "xT": np.ascontiguousarray(xp.T),
            "wq": wq_e, "wk": wk_e, "wv": wv_e, "wp": wp_e,
            "w1": w1_e, "w2": w2_e,
            "bq": bq_e, "bk": bk_e, "bv": bv_e, "bp": bp_e,
            "b1": b1_e, "b2": b2_e,
            "mask": _make_mask(half),
            "consts": consts_np,
        })

    res = run_bass_kernel_spmd(nc, in_maps, core_ids=list(range(8)))

    out = np.empty((B, T, C), np.float32)
    for core in range(8):
        b, half = divmod(core, 2)
        oT = res.results[core]["outT"]           # [C, TQ] own cols
        blocks = oT.reshape(C, TQ // P, P)       # local block m
        for m in range(TQ // P):
            out[b, (2 * m + half) * P:(2 * m + half + 1) * P, :] = \
                blocks[:, m, :].T
    return out



# revision 10
# speedup vs baseline: 1.2955x; 1.2955x over previous
"""Trainium2 Bass kernel for a dense transformer block (nn_Block_5360119185758).

B=4, T=2048, C=512, H=8, DH=64, FF=2048, causal attention, fp32 I/O.

Sharding: 8 cores = (batch b, half) pairs; zero collectives. Each core
computes K/V for its full batch but queries/proj/FFN only for its half of
the rows (alternating 128-row blocks, for causal load balance). Columns are
host-permuted so each core's own rows come first -> the device program is
identical across cores (SPMD); all core-dependence lives in the input data.

Precision: activations + weights are fp8e4 (per-tensor scales) so the big
GEMMs (QKV proj, FFN1, FFN2, AV) run in DoubleRow perf mode (2 rows/cyc,
256-deep contraction per instruction). Scores run plain fp8. The attention
output projection runs in bf16 (OT + Wp) for accuracy margin. LayerNorm
stays fp32; all quantization scale factors fold into existing copy/rsqrt/
activation constants so no extra instructions are spent on scaling.
"""

import sys

if "/opt/trn_rl_repo" not in sys.path:
    sys.path.insert(0, "/opt/trn_rl_repo")

import numpy as np
import ml_dtypes

import concourse.bass as bass
import concourse.mybir as mybir
import concourse.tile as tile
from concourse import bacc
from concourse.bass_utils import run_bass_kernel_spmd

F32 = mybir.dt.float32
F32R = mybir.dt.float32r
BF16 = mybir.dt.bfloat16
FP8 = mybir.dt.float8e4
E4NP = ml_dtypes.float8_e4m3
AF = mybir.ActivationFunctionType
ALU = mybir.AluOpType
DR = mybir.MatmulPerfMode.DoubleRow

B, T, C, H, DH, FF = 4, 2048, 512, 8, 64, 4 * 512
P = 128
KC = C // P            # 4 c-chunks
NBLK = T // P          # 16 global t-blocks
TQ = T // 2            # 1024 own rows per core
NJ = TQ // 256         # 4 supertiles of 256 own cols
FC = FF // P           # 16 f-chunks
EPS = 1e-5
NEG = -1.0e9

SX = 40.0              # xhat fp8 scale (both LayerNorms)
SK = 48.0              # K/Q fp8 scale
SV = 48.0              # V fp8 scale; VO ones-column = SV so OT = av/denom
SP = 16.0              # pt = SP*exp(s)
SF = 16.0              # relu-out fp8 scale


def _build_nc(kq_sc, v_sc, relu_sc, f2_sc, phases=3.0):
    nc = bacc.Bacc(None, target_bir_lowering=False)

    xT = nc.dram_tensor("xT", [C, T], F32, kind="ExternalInput")
    wq = nc.dram_tensor("wq", [C, C], FP8, kind="ExternalInput")
    wk = nc.dram_tensor("wk", [C, C], FP8, kind="ExternalInput")
    wv = nc.dram_tensor("wv", [C, C], FP8, kind="ExternalInput")
    wp = nc.dram_tensor("wp", [C, C], BF16, kind="ExternalInput")
    w1 = nc.dram_tensor("w1", [C, FF], FP8, kind="ExternalInput")
    w2 = nc.dram_tensor("w2", [FF, C], FP8, kind="ExternalInput")
    bqd = nc.dram_tensor("bq", [C], F32, kind="ExternalInput")
    bkd = nc.dram_tensor("bk", [C], F32, kind="ExternalInput")
    bpd = nc.dram_tensor("bp", [C], F32, kind="ExternalInput")
    b1d = nc.dram_tensor("b1", [FF], F32, kind="ExternalInput")
    b2d = nc.dram_tensor("b2", [C], F32, kind="ExternalInput")
    maskd = nc.dram_tensor("mask", [4, P, 256], F32, kind="ExternalInput")
    consts = nc.dram_tensor("consts", [P, 160], F32, kind="ExternalInput")
    outT = nc.dram_tensor("outT", [C, TQ], F32, kind="ExternalOutput")

    with tile.TileContext(nc) as tc:
        _emit(nc, tc, xT, wq, wk, wv, wp, w1, w2,
              bqd, bkd, bpd, b1d, b2d, maskd, consts, outT,
              kq_sc, v_sc, relu_sc, f2_sc, phases=phases)
    nc.compile()
    return nc


def _emit(nc, tc, xT, wq, wk, wv, wp, w1, w2,
          bqd, bkd, bpd, b1d, b2d, maskd, consts, outT,
          kq_sc, v_sc, relu_sc, f2_sc, phases=3):
    import contextlib
    ctx = contextlib.ExitStack()
    with ctx:
        res = ctx.enter_context(tc.tile_pool(name="res", bufs=1))

        # --- resident weights ---
        def load_w8(dram, shape, name):
            t = res.tile(shape, FP8, name=name, tag=name)
            nc.sync.dma_start(t[:], dram.rearrange("(kc p) n -> p kc n", p=P))
            return t

        wk_s = load_w8(wk, [P, KC, C], "wk_s")
        wv_s = load_w8(wv, [P, KC, C], "wv_s")
        wq_s = load_w8(wq, [P, KC, C], "wq_s")

        def load_b(dram, n, name):
            t = res.tile([P, n], F32, name=name, tag=name)
            nc.sync.dma_start(t[:], dram.rearrange("(mc p) -> p mc", p=P))
            return t

        bq_s = load_b(bqd, KC, "bq_s")
        bk_s = load_b(bkd, KC, "bk_s")
        bp_s = load_b(bpd, KC, "bp_s")
        b1_s = load_b(b1d, FC, "b1_s")
        b2_s = load_b(b2d, KC, "b2_s")

        mask_s = res.tile([P, 4, 256], F32)
        nc.sync.dma_start(mask_s[:], maskd.rearrange("m p t -> p m t"))

        ones_l = res.tile([P, 1], F32R)
        nc.sync.dma_start(ones_l[:], consts[:, 0:1].bitcast(F32R))
        eps_r = res.tile([1, 1], F32)
        nc.sync.dma_start(eps_r[:], consts[0:1, 1:2])
        lnsp = res.tile([P, 1], F32)
        nc.sync.dma_start(lnsp[:], consts[:, 2:3])

        # --- resident activations ---
        KT = res.tile([P, KC, T], FP8)        # K^T feature-major, all heads
        VO = res.tile([P, H, NBLK, DH], FP8)  # V token-major, per head
        QT = res.tile([P, KC, TQ], FP8)       # Q^T for own cols
        ones8 = res.tile([P, 2, DH], FP8)     # SV block (M=64) for denoms
        nc.vector.memset(ones8[:], SV)

        # ---------------- Phase A: LN1 stats + QKV projections ----------
        with (
            tc.tile_pool(name="pa", bufs=2) as pa,
            tc.tile_pool(name="pa1", bufs=2) as pa1,
            tc.tile_pool(name="ppa", bufs=3, space="PSUM") as ppa,
            tc.tile_pool(name="pst", bufs=2, space="PSUM") as pst,
        ):
            for ts in range(4):          # 512-col slabs of permuted T
                tsl = slice(ts * 512, (ts + 1) * 512)
                xt = pa.tile([P, KC, 512], F32R, tag="xt")
                nc.sync.dma_start(
                    xt[:], xT.rearrange("(kc p) t -> p kc t", p=P)[:, :, tsl]
                    .bitcast(F32R))

                ssum = pst.tile([1, 512], F32, tag="st")
                ssq = pst.tile([1, 512], F32, tag="st")
                for kc in range(KC):
                    nc.tensor.matmul(ssum[:], ones_l[:], xt[:, kc, :],
                                     start=(kc == 0), stop=(kc == KC - 1))
                for kc in range(KC):
                    xsq = pa1.tile([P, 512], F32R, tag="xsq")
                    nc.vector.tensor_mul(xsq[:], xt[:, kc, :], xt[:, kc, :])
                    nc.tensor.matmul(ssq[:], ones_l[:], xsq[:],
                                     start=(kc == 0), stop=(kc == KC - 1))

                mu = pa1.tile([1, 512], F32, tag="mu")
                r = pa1.tile([1, 512], F32, tag="r")
                sd = pa1.tile([1, 512], F32, tag="sd")
                nc.scalar.copy(mu[:], ssum[:])
                nc.vector.tensor_mul(sd[:], mu[:], mu[:])      # mu^2
                nc.vector.tensor_tensor(sd[:], ssq[:], sd[:], ALU.subtract)
                # sd = sqrt((var+eps)/SX^2) so 1/sd = SX*rstd
                nc.scalar.activation(sd[:], sd[:], AF.Sqrt, bias=eps_r[:],
                                     scale=float(1.0 / (SX * SX)))
                nc.vector.reciprocal(r[:], sd[:])

                mub = pa1.tile([P, 512], F32, tag="mub")
                rb = pa1.tile([P, 512], F32, tag="rb")
                nc.gpsimd.partition_broadcast(mub[:], mu[:])
                nc.gpsimd.partition_broadcast(rb[:], r[:])
                xhf = pa.tile([P, KC, 512], F32, tag="xhf")
                xh = pa.tile([P, KC, 512], FP8, tag="xh")
                for kc in range(KC):
                    nc.vector.tensor_tensor(
                        xhf[:, kc, :], xt[:, kc, :], mub[:],
                        ALU.subtract)
                    nc.vector.tensor_tensor(
                        xh[:, kc, :], xhf[:, kc, :], rb[:], ALU.mult)

                # K^T (+ Q^T for own slabs): DoubleRow, lhsT=W, rhs=xhat
                plans = [(wk_s, bk_s, KT[:, :, tsl])]
                if ts < 2:
                    plans.append(
                        (wq_s, bq_s,
                         QT[:, :, slice(ts * 512, ts * 512 + 512)]))
                for w_s, b_s, dst in plans:
                    for mc in range(KC):
                        ps = ppa.tile([P, 512], F32, tag="mmA")
                        for kp in range(2):
                            nc.tensor.matmul(
                                ps[:],
                                w_s[:, 2 * kp:2 * kp + 2,
                                    mc * P:(mc + 1) * P],
                                xh[:, 2 * kp:2 * kp + 2, :],
                                start=(kp == 0), stop=(kp == 1),
                                perf_mode=DR)
                        nc.scalar.activation(
                            dst[:, mc, :], ps[:], AF.Identity,
                            bias=b_s[:, mc:mc + 1], scale=float(kq_sc))

                # V token-major: lhsT=xhat, rhs=Wv  -> [t, (h d)]
                for tm in range(4):
                    ps = ppa.tile([P, 512], F32, tag="mmA")
                    for kp in range(2):
                        nc.tensor.matmul(
                            ps[:],
                            xh[:, 2 * kp:2 * kp + 2, tm * P:(tm + 1) * P],
                            wv_s[:, 2 * kp:2 * kp + 2, :],
                            start=(kp == 0), stop=(kp == 1),
                            perf_mode=DR)
                    nc.vector.tensor_scalar(
                        VO[:, :, ts * 4 + tm, :],
                        ps[:].rearrange("p (h d) -> p h d", h=H),
                        float(v_sc), None, ALU.mult)

        # phase-B-only weights queued after the phase-A DMA stream
        wp_s = res.tile([P, KC, C], BF16, name="wp_s", tag="wp_s")
        nc.sync.dma_start(wp_s[:], wp.rearrange("(kc p) n -> p kc n", p=P))
        w1_s = load_w8(w1, [P, KC, FF], "w1_s")
        w2_s = res.tile([P, FC, C], FP8, name="w2_s", tag="w2_s")
        nc.sync.dma_start(w2_s[:], w2.rearrange("(fc p) n -> p fc n", p=P))
        if phases < 2:
            return
        # ---------------- Phase B: attention + proj + LN2 + FFN ---------
        with (
            tc.tile_pool(name="pb", bufs=1) as pb,
            tc.tile_pool(name="pb2", bufs=1) as pb2,
            tc.tile_pool(name="pbq", bufs=1) as pbq,
            tc.tile_pool(name="pbr", bufs=1) as pbr,
            tc.tile_pool(name="ppt", bufs=3) as ppt,
            tc.tile_pool(name="pb1", bufs=2) as pb1,
            tc.tile_pool(name="prl", bufs=3) as prl,
            tc.tile_pool(name="pps", bufs=8, space="PSUM") as pps,
        ):
            for j in range(NJ):
                jsl = slice(j * 256, (j + 1) * 256)
                nown = 2 * j + 2          # own s-blocks (last 2 masked)
                sblocks = [(m, (0 if m == nown - 2 else 1 if m == nown - 1
                                else None))
                           for m in range(nown)]
                sblocks += [(8 + m, (2 if m == nown - 2 else
                                     3 if m == nown - 1 else None))
                            for m in range(nown)]
                ns = len(sblocks)

                x_own = pb.tile([P, KC, 256], F32, tag="xo")
                nc.sync.dma_start(
                    x_own[:],
                    xT.rearrange("(kc p) t -> p kc t", p=P)[:, :, jsl])

                qe = pbq.tile([P, KC, 256], FP8, tag="qe")
                qo = pbq.tile([P, KC, 256], FP8, tag="qo")
                nc.vector.tensor_copy(qe[0:64, :, :], QT[0:64, :, jsl])
                nc.vector.memset(qe[64:P, :, :], 0.0)
                nc.vector.tensor_copy(qo[64:P, :, :], QT[64:P, :, jsl])
                nc.vector.memset(qo[0:64, :, :], 0.0)
                OT = pb2.tile([P, KC, 256], BF16, tag="ot")
                for hp in range(H // 2):      # head pairs share sc tiles
                    av = [None, None]
                    dn = None
                    pendp = None          # (ptp, k0) pair awaiting AV-DR
                    ptp = None
                    for ki in range(ns):
                        sb, mi = sblocks[ki]
                        ssl = slice(sb * P, (sb + 1) * P)
                        sc = pps.tile([P, 512], F32, tag="u", name="sc")
                        for o in range(2):   # even/odd head of pair
                            nc.tensor.matmul(
                                sc[:, o * 256:(o + 1) * 256],
                                KT[:, hp, ssl],
                                (qe if o == 0 else qo)[:, hp, :],
                                start=True, stop=True)
                        if mi is not None:
                            m2 = mask_s[:, mi, None, :] \
                                .to_broadcast((P, 2, 256))
                            nc.vector.tensor_tensor(
                                sc[:].rearrange("p (b t) -> p b t", b=2),
                                sc[:].rearrange("p (b t) -> p b t", b=2),
                                m2, ALU.add)
                        if ki % 2 == 0:
                            ptp = ppt.tile([P, 2, 2, 256], FP8, tag="pt",
                                           name="pt")
                        nc.scalar.activation(
                            ptp[:, ki % 2, :, :]
                            .rearrange("p o t -> p (o t)"),
                            sc[:], AF.Exp,
                            scale=float(1.0 / (SK * SK * np.sqrt(DH))),
                            bias=lnsp[:])
                        for o in range(2):
                            if av[o] is None:
                                av[o] = pps.tile([P, 512], F32,
                                                 tag="u", name="av")
                        if dn is None:
                            dn = pps.tile([P, 512], F32, tag="u", name="dn")
                        if ki % 2 == 1:
                            if pendp is not None:
                                pptp, pk0 = pendp
                                psb = sblocks[pk0][0]
                                for o in range(2):
                                    nc.tensor.matmul(
                                        av[o][0:DH, 0:256],
                                        VO[:, 2 * hp + o, psb:psb + 2, :],
                                        pptp[:, :, o, :],
                                        start=(pk0 == 0), stop=False,
                                        perf_mode=DR)

                                nc.tensor.matmul(
                                    dn[0:DH, :],
                                    ones8[:],
                                    pptp[:, :, :, :]
                                    .rearrange("p a o t -> p a (o t)"),
                                    start=(pk0 == 0), stop=False,
                                    perf_mode=DR)
                            pendp = (ptp, ki - 1)
                    pptp, pk0 = pendp
                    psb = sblocks[pk0][0]
                    for o in range(2):
                        nc.tensor.matmul(
                            av[o][0:DH, 0:256],
                            VO[:, 2 * hp + o, psb:psb + 2, :],
                            pptp[:, :, o, :],
                            start=(pk0 == 0), stop=True,
                            perf_mode=DR)

                    nc.tensor.matmul(
                        dn[0:DH, :],
                        ones8[:],
                        pptp[:, :, :, :].rearrange("p a o t -> p a (o t)"),
                        start=(pk0 == 0), stop=True,
                        perf_mode=DR)
                    for o in range(2):
                        rec = pbr.tile([1, 256], F32, tag="rec")
                        nc.vector.reciprocal(
                            rec[:], dn[0:1, o * 256:(o + 1) * 256])
                        recb = pb1.tile([DH, 256], F32, tag="recb")
                        nc.gpsimd.partition_broadcast(recb[:], rec[:])
                        h0 = 64 * o
                        nc.vector.tensor_tensor(
                            OT[h0:h0 + 64, hp, :], av[o][0:DH, 0:256],
                            recb[:], ALU.mult)

                if phases < 2.2:
                    continue
                # proj (bf16) + bp + residual -> resid (feature-major)
                resid = pb2.tile([P, KC, 256], F32R, tag="resid")
                for cc in range(KC):
                    ps = pps.tile([P, 512], F32, tag="u", name="ps")
                    for kc in range(KC):
                        nc.tensor.matmul(
                            ps[:, 0:256], wp_s[:, kc, cc * P:(cc + 1) * P],
                            OT[:, kc, :],
                            start=(kc == 0), stop=(kc == KC - 1))
                    nc.vector.scalar_tensor_tensor(
                        resid[:, cc, :], ps[:, 0:256],
                        bp_s[:, cc:cc + 1], x_own[:, cc, :],
                        ALU.add, ALU.add)

                # LN2 stats
                ssum = pps.tile([1, 512], F32, tag="u", name="ssum")
                ssq = pps.tile([1, 512], F32, tag="u", name="ssq")
                for kc in range(KC):
                    nc.tensor.matmul(ssum[0:1, 0:256], ones_l[:],
                                     resid[:, kc, :],
                                     start=(kc == 0), stop=(kc == KC - 1))
                for kc in range(KC):
                    xsq = pb1.tile([P, 256], F32R, tag="xsq2")
                    nc.vector.tensor_mul(xsq[:], resid[:, kc, :],
                                         resid[:, kc, :])
                    nc.tensor.matmul(ssq[0:1, 0:256], ones_l[:], xsq[:],
                                     start=(kc == 0), stop=(kc == KC - 1))
                mu = pbr.tile([1, 256], F32, tag="mu2")
                r = pbr.tile([1, 256], F32, tag="r2")
                sd = pbr.tile([1, 256], F32, tag="sd2")
                nc.scalar.copy(mu[:], ssum[0:1, 0:256])
                nc.vector.tensor_mul(sd[:], mu[:], mu[:])
                nc.vector.tensor_tensor(sd[:], ssq[0:1, 0:256], sd[:],
                                        ALU.subtract)
                nc.scalar.activation(sd[:], sd[:], AF.Sqrt, bias=eps_r[:],
                                     scale=float(1.0 / (SX * SX)))
                nc.vector.reciprocal(r[:], sd[:])

                if phases < 2.5:
                    continue
                mub2 = pb1.tile([P, 256], F32, tag="mub2")
                rb2 = pb1.tile([P, 256], F32, tag="rb2")
                nc.gpsimd.partition_broadcast(mub2[:], mu[:])
                nc.gpsimd.partition_broadcast(rb2[:], r[:])
                xh2f = pb2.tile([P, KC, 256], F32, tag="xh2f")
                xh2 = pb2.tile([P, KC, 256], FP8, tag="xh2")
                for kc in range(KC):
                    nc.vector.tensor_tensor(
                        xh2f[:, kc, :], resid[:, kc, :],
                        mub2[:], ALU.subtract)
                    nc.vector.tensor_tensor(
                        xh2[:, kc, :], xh2f[:, kc, :], rb2[:], ALU.mult)

                # FFN (fp8 DoubleRow both layers, resident W2)
                if phases < 2.8:
                    continue
                f2t = [pps.tile([P, 512], F32, tag="u", name="f2")
                       for _ in range(KC)]
                f2sl = [t[:, 0:256] for t in f2t]
                rlp = None
                pend = None               # (rlp, fp_) awaiting ffn2
                for fc in range(FC):
                    ps = pps.tile([P, 512], F32, tag="u", name="ps")
                    for kp in range(2):
                        nc.tensor.matmul(
                            ps[:, 0:256],
                            w1_s[:, 2 * kp:2 * kp + 2,
                                 fc * P:(fc + 1) * P],
                            xh2[:, 2 * kp:2 * kp + 2, :],
                            start=(kp == 0), stop=(kp == 1),
                            perf_mode=DR)
                    if fc % 2 == 0:
                        rlp = prl.tile([P, 2, 256], FP8, tag="rl")
                    nc.scalar.activation(rlp[:, fc % 2, :], ps[:, 0:256],
                                         AF.Relu,
                                         bias=b1_s[:, fc:fc + 1],
                                         scale=float(relu_sc))
                    if fc % 2 == 1:
                        if pend is not None:
                            prlp, pfp = pend
                            for cc in range(KC):
                                nc.tensor.matmul(
                                    f2sl[cc],
                                    w2_s[:, 2 * pfp:2 * pfp + 2,
                                         cc * P:(cc + 1) * P],
                                    prlp[:],
                                    start=(pfp == 0), stop=False,
                                    perf_mode=DR)
                        pend = (rlp, fc // 2)
                prlp, pfp = pend
                for cc in range(KC):
                    nc.tensor.matmul(
                        f2sl[cc],
                        w2_s[:, 2 * pfp:2 * pfp + 2, cc * P:(cc + 1) * P],
                        prlp[:],
                        start=(pfp == 0), stop=True,
                        perf_mode=DR)

                ot = pb2.tile([P, KC, 256], F32, tag="outb")
                for cc in range(KC):
                    nc.vector.tensor_scalar(
                        ot[:, cc, :], f2sl[cc],
                        float(f2_sc), b2_s[:, cc:cc + 1],
                        ALU.mult, ALU.add)
                    nc.vector.tensor_tensor(
                        ot[:, cc, :], ot[:, cc, :],
                        resid[:, cc, :], ALU.add)
                nc.sync.dma_start(
                    outT.rearrange("(kc p) t -> p kc t", p=P)[:, :, jsl],
                    ot[:])


_NC_CACHE = {}


def _get_nc(scales):
    key = tuple(round(float(s), 9) for s in scales)
    if key not in _NC_CACHE:
        _NC_CACHE[key] = _build_nc(*scales)
    return _NC_CACHE[key]


def _perm_blocks(half):
    return list(range(half, NBLK, 2)) + list(range(1 - half, NBLK, 2))


def _make_mask(half):
    m = np.zeros((4, P, 256), np.float32)
    s_in = np.arange(P)[:, None]
    t_in = np.arange(256)[None, :] % P
    n = np.arange(256)[None, :] // P        # own t-block 0/1 (relative)
    g_t = 2 * n + half
    for mi in range(4):
        if mi < 2:
            g_s = 2 * mi + half             # own s-block (relative)
        else:
            g_s = 2 * (mi - 2) + 1 - half   # partner s-block
        allowed = (g_s * P + s_in) <= (g_t * P + t_in)
        m[mi][~allowed] = NEG
    return m


def _q8(a, sc):
    return np.clip(np.asarray(a, np.float32) * sc, -240, 240).astype(E4NP)


def kernel(x, ln1_g, ln1_b, Wq, bq, Wk, bk, Wv, bv, Wp, bp,
           ln2_g, ln2_b, W1, b1, W2, b2):
    x = np.asarray(x, np.float32)
    f = lambda a: np.asarray(a, np.float32)
    ln1_g, ln1_b, ln2_g, ln2_b = f(ln1_g), f(ln1_b), f(ln2_g), f(ln2_b)
    Wqf = f(Wq).transpose(1, 0, 2).reshape(C, C)
    Wkf = f(Wk).transpose(1, 0, 2).reshape(C, C)
    Wvf = f(Wv).transpose(1, 0, 2).reshape(C, C)
    wq_e = np.ascontiguousarray(ln1_g[:, None] * Wqf)
    wk_e = np.ascontiguousarray(ln1_g[:, None] * Wkf)
    wv_e = np.ascontiguousarray(ln1_g[:, None] * Wvf)
    bq_e = f(bq).reshape(C) + ln1_b @ Wqf
    bk_e = f(bk).reshape(C) + ln1_b @ Wkf
    bv_e = f(bv).reshape(C) + ln1_b @ Wvf
    w1_e = np.ascontiguousarray(ln2_g[:, None] * f(W1))
    b1_e = f(b1) + ln2_b @ f(W1)
    wp_e, bp_e, w2_e, b2_e = f(Wp), f(bp), f(W2), f(b2)
    bp_e = bp_e + bv_e @ wp_e      # V bias folded into proj bias

    sWk = 240.0 / max(np.abs(wq_e).max(), np.abs(wk_e).max())
    sWq = sWk
    sWv = 240.0 / np.abs(wv_e).max()
    sW1 = 240.0 / np.abs(w1_e).max()
    sW2 = 240.0 / np.abs(w2_e).max()
    wq8, wk8, wv8 = _q8(wq_e, sWq), _q8(wk_e, sWk), _q8(wv_e, sWv)
    w18, w28 = _q8(w1_e, sW1), _q8(w2_e, sW2)
    wpb = np.asarray(wp_e, ml_dtypes.bfloat16)

    # baked device scale constants
    kq_sc = SK / (sWk * SX)        # K/Q copy: KT8 = raw*kq_sc + SK*bk_e
    v_sc = SV / (sWv * SX)
    relu_sc = SF / (sW1 * SX)
    f2_sc = 1.0 / (sW2 * SF)

    nc = _get_nc((kq_sc, v_sc, relu_sc, f2_sc))
    consts_np = np.ones((P, 160), np.float32)
    consts_np[:, 0] = 1.0 / C
    consts_np[0, 1] = EPS / (SX * SX)
    consts_np[:, 2] = np.log(SP)
    in_maps = []
    for core in range(8):
        b_, half = divmod(core, 2)
        pb_ = _perm_blocks(half)
        xp = x[b_].reshape(NBLK, P, C)[pb_].reshape(T, C)
        in_maps.append({
            "xT": np.ascontiguousarray(xp.T),
            "wq": wq8, "wk": wk8, "wv": wv8, "wp": wpb,
            "w1": w18, "w2": w28,
            "bq": SK * bq_e, "bk": SK * bk_e, "bp": bp_e,
            "b1": SF * b1_e, "b2": b2_e,
            "mask": _make_mask(half),
            "consts": consts_np,
        })

    res = run_bass_kernel_spmd(nc, in_maps, core_ids=list(range(8)))

    out = np.empty((B, T, C), np.float32)
    for core in range(8):
        b_, half = divmod(core, 2)
        oT = res.results[core]["outT"]           # [C, TQ] own cols
        blocks = oT.reshape(C, TQ // P, P)       # local block m
        for m in range(TQ // P):
            out[b_, (2 * m + half) * P:(2 * m + half + 1) * P, :] = \
                blocks[:, m, :].T
    return out
